# revision 5
# baseline (speedup 1.0000x reference)
"""PaDiM AnomalyMapGenerator kernel for 8 TRN2 NeuronCores.

Pipeline (per the reference):
  1. delta = embedding - mean                                   [B, C, HW]
  2. q[n, b] = delta_nb^T @ inv_cov_n @ delta_nb  (per pixel)   [HW, B]
  3. dist = sqrt(max(q, 0)) -> score maps                       [B, 56, 56]
  4. bilinear 4x upsample + 33x33 gaussian blur (reflect)       [B, 1, 224, 224]

Stage 4 is a fixed linear operator, so it collapses to out_b = M @ S_b @ M^T
with M = Blur(224x224, reflect) @ Resize(224x56) precomputed on host.

Sharding: pixels (HW) are split across the 8 cores for stage 1-3 (the 500MB
inv_covariance read dominates); a tiny AllToAll (50KB/core) redistributes the
score maps so each core post-processes 4 images (B-sharding) for stage 4.
"""

import os
import sys

import numpy as np

for _p in ("/opt/trn_rl_repo", "/root/.axon_site/_ro/trn_rl_repo"):
    if os.path.isdir(_p) and _p not in sys.path:
        sys.path.append(_p)

import concourse.bacc as bacc
import concourse.mybir as mybir
import concourse.tile as tile
from concourse.bass_utils import run_bass_kernel_spmd
from concourse.masks import make_identity

F32 = mybir.dt.float32

B, C, H, W = 32, 200, 56, 56
HW = H * W                 # 3136
NCORES = 8
NL = HW // NCORES          # 392 pixels per core
KA, KB = 128, 72           # contraction (channel) split
IMG = 224
SIGMA = 4.0
KS = 33
PAD = (KS - 1) // 2

GRP = 4                    # pixels per matmul group (PE column tiling)
BLK = 8                    # pixels per inv_cov DMA block
NBLK = NL // BLK           # 49
GPB = BLK // GRP           # groups per block
NGRP = NL // GRP           # 98
IPC = B // NCORES          # images per core


def _mt_matrix():
    """M^T [56, 224] for out_b = M @ S_b @ M^T == blur(upsample(S_b))."""
    # Bilinear resize weights, jax.image.resize convention (half-pixel centers,
    # triangle kernel, per-row renormalization).  Upsampling -> no antialiasing.
    scale = IMG / H
    u = (np.arange(IMG, dtype=np.float64) + 0.5) / scale - 0.5
    k = np.arange(H, dtype=np.float64)
    wts = np.maximum(0.0, 1.0 - np.abs(u[:, None] - k[None, :]))
    R = wts / wts.sum(axis=1, keepdims=True)          # [224, 56]
    # Gaussian blur with reflect padding as a dense matrix.
    x = np.arange(KS, dtype=np.float64) - (KS - 1) / 2.0
    g = np.exp(-(x * x) / (2.0 * SIGMA * SIGMA))
    g = g / g.sum()
    Bm = np.zeros((IMG, IMG), dtype=np.float64)
    for i in range(IMG):
        for j in range(KS):
            t = i + j - PAD
            if t < 0:
                t = -t
            if t >= IMG:
                t = 2 * IMG - 2 - t
            Bm[i, t] += g[j]
    M = Bm @ R                                         # [224, 56]
    return np.ascontiguousarray(M.T, dtype=np.float32)  # [56, 224]


def build():
    nc = bacc.Bacc("TRN2", target_bir_lowering=False, debug=False,
                   num_devices=NCORES)
    emb = nc.dram_tensor("embedding", [B, C, NL], F32, kind="ExternalInput").ap()
    mean = nc.dram_tensor("mean", [C, NL], F32, kind="ExternalInput").ap()
    icov = nc.dram_tensor("inv_covariance", [NL, C, C], F32,
                          kind="ExternalInput").ap()
    mt = nc.dram_tensor("mt", [H, IMG], F32, kind="ExternalInput").ap()
    outp = nc.dram_tensor("out", [IPC, IMG, IMG], F32, kind="ExternalOutput").ap()

    emb_t = emb.rearrange("b i n -> i b n")     # [C, B, NL]
    icov_t = icov.rearrange("n i j -> i n j")   # [C, NL, C]

    with tile.TileContext(nc) as tc:
        with (
            tc.tile_pool(name="const", bufs=1) as cpool,
            tc.tile_pool(name="dt", bufs=3) as dtpool,
            tc.tile_pool(name="ic", bufs=3) as icpool,
            tc.tile_pool(name="scr", bufs=2) as scrpool,
            tc.tile_pool(name="post", bufs=2) as postpool,
            tc.tile_pool(name="psmd", bufs=3, space="PSUM") as psmd,
            tc.tile_pool(name="psd", bufs=3, space="PSUM") as psd,
            tc.tile_pool(name="pspost", bufs=1, space="PSUM") as pspost,
            tc.tile_pool(name="dram", bufs=1, space="DRAM") as dram,
        ):
            # ---- one-time loads --------------------------------------------
            Ea = cpool.tile([KA, B, NL], F32, tag="Ea")
            Eb = cpool.tile([KB, B, NL], F32, tag="Eb")
            ma = cpool.tile([KA, NL], F32, tag="ma")
            mb = cpool.tile([KB, NL], F32, tag="mb")
            mts = cpool.tile([H, IMG], F32, tag="mts")
            id128 = cpool.tile([KA, KA], F32, tag="id128")
            id72 = cpool.tile([KB, KB], F32, tag="id72")
            Q = cpool.tile([128, NGRP], F32, tag="Q")
            Qs = cpool.tile([128, NGRP], F32, tag="Qs")
            dist_sb = cpool.tile([B, NL], F32, tag="dist")

            nc.sync.dma_start(Ea[:], emb_t[0:KA])
            nc.sync.dma_start(Eb[:], emb_t[KA:C])
            nc.sync.dma_start(ma[:], mean[0:KA])
            nc.sync.dma_start(mb[:], mean[KA:C])
            nc.sync.dma_start(mts[:], mt)
            make_identity(nc, id128[:])
            make_identity(nc, id72[:])

            # ---- per-pixel mahalanobis -------------------------------------
            for blk in range(NBLK):
                n0 = blk * BLK
                dTa = dtpool.tile([KA, B, BLK], F32, tag="dta")
                dTb = dtpool.tile([KB, B, BLK], F32, tag="dtb")
                nc.vector.tensor_sub(
                    dTa[:], Ea[:, :, n0:n0 + BLK],
                    ma[:, n0:n0 + BLK].unsqueeze(1).broadcast_to((KA, B, BLK)))
                nc.vector.tensor_sub(
                    dTb[:], Eb[:, :, n0:n0 + BLK],
                    mb[:, n0:n0 + BLK].unsqueeze(1).broadcast_to((KB, B, BLK)))

                ica = icpool.tile([KA, BLK, C], F32, tag="ica")
                icb = icpool.tile([KB, BLK, C], F32, tag="icb")
                nc.sync.dma_start(ica[:], icov_t[0:KA, n0:n0 + BLK])
                nc.sync.dma_start(icb[:], icov_t[KA:C, n0:n0 + BLK])

                for gl in range(GPB):
                    g = blk * GPB + gl
                    # PSUM tiles use the full 2KB bank row (512 f32) so each
                    # pixel's 32-partition slice is a distinct HW zero region.
                    ps_md = psmd.tile([128, 512], F32, tag="psmd")
                    ps_d = psd.tile([128, 512], F32, tag="psd")
                    for p in range(GRP):
                        lp = gl * GRP + p
                        wa = dTa[:, :, lp]          # [KA, B] strided
                        wb = dTb[:, :, lp]          # [KB, B]
                        tp = (0, 32 * p)
                        o = ps_md[32 * p:32 * p + 32, 0:C]
                        # md = delta^T IC   (K split, accumulate)
                        nc.tensor.matmul(o, wa, ica[:, lp, :],
                                         start=True, stop=False, tile_position=tp)
                        nc.tensor.matmul(o, wb, icb[:, lp, :],
                                         start=False, stop=True, tile_position=tp)
                        # delta copied to PSUM in [b, i] layout via identity mm
                        # (one accumulation group, disjoint column ranges)
                        nc.tensor.matmul(ps_d[32 * p:32 * p + 32, 0:KA], wa,
                                         id128[:], start=True, stop=False,
                                         tile_position=tp)
                        nc.tensor.matmul(ps_d[32 * p:32 * p + 32, KA:C], wb,
                                         id72[:], start=False, stop=True,
                                         tile_position=tp)
                    d_sb = scrpool.tile([128, C], F32, tag="dsb")
                    nc.scalar.copy(d_sb[:], ps_d[:, 0:C])
                    scr = scrpool.tile([128, C], F32, tag="scr")
                    # q = sum_j md * delta (InstTensorTensorReduce crashes this
                    # runtime, so multiply + reduce as two DVE ops)
                    nc.vector.tensor_mul(scr[:], ps_md[:, 0:C], d_sb[:])
                    nc.vector.reduce_sum(Q[:, g:g + 1], scr[:],
                                         axis=mybir.AxisListType.X)

            # ---- dist = sqrt(relu(q)); relayout [(p,b), g] -> [b, 4g+p] ----
            nc.vector.tensor_scalar_max(Q[:], Q[:], 0.0)
            nc.scalar.sqrt(Qs[:], Q[:])
            dview = dist_sb[:].rearrange("b (g p) -> p b g", p=GRP)
            for p in range(GRP):
                nc.sync.dma_start(dview[p], Qs[32 * p:32 * p + 32, :])

            # ---- AllToAll: pixel-shard -> image-shard ----------------------
            dist_dram = dram.tile([B, NL], F32, tag="dist_dram")
            a2a = dram.tile([B, NL], F32, tag="a2a")
            s_dram = dram.tile([IPC, H, W], F32, tag="s_dram")
            nc.sync.dma_start(dist_dram[:], dist_sb[:])
            nc.gpsimd.collective_compute(
                "AllToAll", mybir.AluOpType.bypass,
                replica_groups=[list(range(NCORES))],
                ins=[dist_dram[:].opt()],
                outs=[a2a[:].opt()],
            )
            # s_dram[t, 7*sc+rl, c] = a2a[4*sc + t, 56*rl + c]
            nc.sync.dma_start(
                s_dram[:].rearrange("t (sc rl) c -> t sc rl c", sc=NCORES),
                a2a[:].rearrange("(sc t) (rl c) -> t sc rl c", t=IPC, c=W))

            # ---- out_b = M @ S_b @ M^T  (as two matmuls vs M^T) ------------
            for t in range(IPC):
                sk = postpool.tile([H, W], F32, tag="sk")
                nc.sync.dma_start(sk[:], s_dram[t])
                psw = pspost.tile([H, 512], F32, tag="psw")
                # W = S^T M^T  ( = (M S)^T )
                nc.tensor.matmul(psw[:, 0:IMG], sk[:], mts[:],
                                 start=True, stop=True)
                wsb = postpool.tile([H, IMG], F32, tag="wsb")
                nc.scalar.copy(wsb[:], psw[:, 0:IMG])
                for h2 in range(2):
                    pso = pspost.tile([112, 512], F32, tag="pso")
                    # out rows = W^T M^T
                    nc.tensor.matmul(pso[:, 0:IMG],
                                     wsb[:, 112 * h2:112 * h2 + 112],
                                     mts[:], start=True, stop=True)
                    osb = postpool.tile([112, IMG], F32, tag="osb")
                    nc.scalar.copy(osb[:], pso[:, 0:IMG])
                    nc.sync.dma_start(outp[t, 112 * h2:112 * h2 + 112, :],
                                      osb[:])

    nc.compile()
    return nc


_NC = None


def _get_nc():
    global _NC
    if _NC is None:
        _NC = build()
    return _NC


def make_in_maps(embedding, mean, inv_covariance):
    emb = np.ascontiguousarray(
        np.asarray(embedding, dtype=np.float32).reshape(B, C, HW))
    mean = np.asarray(mean, dtype=np.float32)
    icov = np.asarray(inv_covariance, dtype=np.float32)
    mt = _mt_matrix()
    in_maps = []
    for i in range(NCORES):
        sl = slice(i * NL, (i + 1) * NL)
        in_maps.append({
            "embedding": np.ascontiguousarray(emb[:, :, sl]),
            "mean": np.ascontiguousarray(mean[:, sl]),
            "inv_covariance": np.ascontiguousarray(icov[sl]),
            "mt": mt,
        })
    return in_maps


def run(embedding, mean, inv_covariance, trace=False):
    nc = _get_nc()
    in_maps = make_in_maps(embedding, mean, inv_covariance)
    res = run_bass_kernel_spmd(nc, in_maps, list(range(NCORES)), trace=trace)
    outs = [res.results[i]["out"] for i in range(NCORES)]
    full = np.concatenate(outs, axis=0).reshape(B, 1, IMG, IMG)
    return np.ascontiguousarray(full, dtype=np.float32), res


def kernel(embedding, mean, inv_covariance, covariance=None):
    out, _ = run(embedding, mean, inv_covariance, trace=False)
    return out
